# revision 29
# baseline (speedup 1.0000x reference)
"""Trainium2 Bass kernel for nn_L2MLoRAqkv (MoE-routed LoRA QKV projection).

Math (per batch b, expert i = idx[b,0]):
    qkv = x @ W.T + bias
    qkv[:, :D]  += (x @ A_q[i]) @ B_q[i] * SCALE
    qkv[:, -D:] += (x @ A_v[i]) @ B_v[i] * SCALE

Strategy: data-parallel over the batch dim (1 batch per NeuronCore, 8 cores).
The rank-8 LoRA update is folded into the projection weight on the host, so
each core runs one dense GEMM  Y[4096, 3072] = X[4096, 1024] @ W_eff + bias.

Precision: hybrid K-split.  6 of the 8 k-slices (768 of K=1024) run in
bf16; the last k-pair (256) runs as a single fp8(e4m3) DoubleRow matmul,
which processes 2 k-slices per instruction (2x MAC rate).  That is 7
matmul-units per output tile instead of 8 (-12.5% PE time) at a measured
1.60e-2 relative error (gate 2e-2).  Scale consistency: the fp8 operands
are scaled by power-of-2 factors sx8*sw8 = s8; the bf16 W is pre-scaled by
s8 on the host, so one PSUM group accumulates everything at scale s8 and
the drain applies 1/s8 (per-partition scalar) plus bias in a single fused
op, alternating DVE / Act (output computed transposed: N on partitions).

PE schedule is weight-stationary with a post-legalize pass that deletes
LDWEIGHTS instructions whose weights the PE array already holds.  X and W
are fully SBUF-resident; warmup interleaves n=0/n=1 t-outer so the PE
never outruns the X DMA; the last n-slice drains/stores per t-chunk to
shorten the tail.  The host un-transposes at the end.
"""

import math
import os
import sys

import numpy as np
import ml_dtypes

for _p in ("/opt/trn_rl_repo",):
    if _p not in sys.path and os.path.isdir(_p):
        sys.path.insert(0, _p)

B = 8          # batches == cores
T = 4096       # tokens per batch
D = 1024       # model dim (contraction K)
N3 = 3072      # qkv output dim
P = 128        # SBUF partitions
KT = D // P    # 8 k-slices
KB = 6         # k-slices computed in bf16 (the rest: one fp8 DoubleRow pair)
TT = T // 512  # 8 token chunks of 512 (PSUM bank free size)
NN = N3 // P   # 24 output n-slices of 128 (partition dim of transposed out)
SCALE = 8.0 / 8.0

BF16 = ml_dtypes.bfloat16
E4 = ml_dtypes.float8_e4m3
FP8_TARGET = 224.0  # scale target under e4m3 max 240

_NC_CACHE = {}


def _install_ldweights_dedup():
    """Patch tile_legalize to drop LDWEIGHTS whose weights AP matches the
    one already loaded in the PE array (weight-stationary reuse)."""
    import concourse.tile as tile
    from concourse import mybir

    if getattr(tile, "_ldw_dedup_installed", False):
        return
    orig = tile.tile_legalize

    def _sig(inst):
        ap = inst.ins[0]
        try:
            s = ap.pretty_str()
        except Exception:
            s = str(ap)
        return (s, str(inst.perf_mode), str(inst.is_transpose))

    def legalize_dedup(ordered, nc):
        ordered = orig(ordered, nc)
        remap = {}
        # The tile program here is straight-line (blocks fall through in
        # emission order), so the loaded-weights signature carries across
        # block boundaries.
        cur = None
        for bb, insts in ordered.items():
            out = []
            pending = []
            for inst in insts:
                if isinstance(inst, mybir.InstLdweights):
                    sig = _sig(inst)
                    if sig == cur:
                        pending.append(inst)
                        continue
                    cur = sig
                    out.append(inst)
                else:
                    if pending and isinstance(inst, mybir.InstMatmult):
                        for p in pending:
                            inst.merge_dependencies_from(p)
                            remap[p.name] = inst.name
                        pending = []
                    out.append(inst)
            assert not pending, "deleted ldweights with no following matmul"
            ordered[bb] = out
        if remap:
            for insts in ordered.values():
                for inst in insts:
                    inst.remap_dependency_names(remap)
        return ordered

    tile.tile_legalize = legalize_dedup
    tile._ldw_dedup_installed = True


def _build():
    _install_ldweights_dedup()
    import concourse.tile as tile
    from concourse import bacc, mybir

    bf = mybir.dt.bfloat16
    f8 = mybir.dt.float8e4
    f32 = mybir.dt.float32
    DR = mybir.MatmulPerfMode.DoubleRow
    Identity = mybir.ActivationFunctionType.Identity
    mult = mybir.AluOpType.mult
    add = mybir.AluOpType.add

    nc = bacc.Bacc(
        "TRN2",
        target_bir_lowering=False,
        debug=False,
        enable_asserts=False,
        num_devices=B,
    )
    # X: [partition, k-slice, token].  W: n-block-major [n-slice, partition,
    # k-slice, ncol] so per-n-slice loads are contiguous descriptors.
    xb_d = nc.dram_tensor("xb", [P, KB, T], bf, kind="ExternalInput").ap()
    x8_d = nc.dram_tensor("x8", [P, 2, T], f8, kind="ExternalInput").ap()
    wb_d = nc.dram_tensor("wb", [NN, P, KB, P], bf, kind="ExternalInput").ap()
    w8_d = nc.dram_tensor("w8", [NN, P, 2, P], f8, kind="ExternalInput").ap()
    biasv_d = nc.dram_tensor("biasv", [P, NN], f32, kind="ExternalInput").ap()
    cvec_d = nc.dram_tensor("cvec", [P, 1], f32, kind="ExternalInput").ap()
    yt = nc.dram_tensor("yt", [N3, T], f32, kind="ExternalOutput").ap()

    with tile.TileContext(nc) as tc:
        with tc.tile_pool(name="const", bufs=1) as const_pool, \
             tc.tile_pool(name="rows", bufs=4) as row_pool, \
             tc.tile_pool(name="ps", bufs=8, space="PSUM") as psum_pool:

            cvec_sb = const_pool.tile([P, 1], f32)
            biasv_sb = const_pool.tile([P, NN], f32)
            xb_sb = const_pool.tile([P, KB, T], bf)
            x8_sb = const_pool.tile([P, 2, T], f8)
            wb_sb = const_pool.tile([P, NN, KB, P], bf)
            w8_sb = const_pool.tile([P, NN, 2, P], f8)

            def load_wn(n):
                nc.sync.dma_start(wb_sb[:, n, :, :], wb_d[n, :, :, :])
                nc.sync.dma_start(w8_sb[:, n, :, :], w8_d[n, :, :, :])

            # Load order = first-use order; drain constants on the idle
            # scalar ring, n=1 weights folded in after the first X chunk.
            def load_x(ts):
                nc.sync.dma_start(xb_sb[:, :, ts], xb_d[:, :, ts])
                nc.sync.dma_start(x8_sb[:, :, ts], x8_d[:, :, ts])

            nc.scalar.dma_start(cvec_sb[:], cvec_d[:])
            nc.scalar.dma_start(biasv_sb[:], biasv_d[:])
            load_wn(0)
            for t in range(TT):
                load_x(slice(t * 512, (t + 1) * 512))
                if t == 0:
                    load_wn(1)
            for n in range(2, NN):
                load_wn(n)

            def mm_piece(n, ts, pdst):
                """All K for one token sub-range into a PSUM sub-region."""
                for k in range(KB):
                    nc.tensor.matmul(
                        pdst,
                        lhsT=wb_sb[:, n, k, :],
                        rhs=xb_sb[:, k, ts],
                        start=(k == 0),
                        stop=False,
                    )
                nc.tensor.matmul(
                    pdst,
                    lhsT=w8_sb[:, n, :, :],
                    rhs=x8_sb[:, :, ts],
                    start=False,
                    stop=True,
                    perf_mode=DR,
                )

            def mm_group(n, t, ps):
                """All K for one (n, t) tile into one PSUM group."""
                mm_piece(n, slice(t * 512, (t + 1) * 512), ps)

            def drain(n, t, ps, row):
                dst = row[:, t * 512 : (t + 1) * 512]
                bias_ap = biasv_sb[:, n : n + 1]
                if (n + t) % 2 == 0:
                    nc.vector.tensor_scalar(
                        dst, ps, cvec_sb[:, 0:1], bias_ap, mult, add
                    )
                else:
                    nc.scalar.activation(
                        dst, ps, Identity,
                        bias=bias_ap, scale=cvec_sb[:, 0:1],
                    )

            def store_row(n, row, nchunks):
                per = T // nchunks
                for c in range(nchunks):
                    cs = slice(c * per, (c + 1) * per)
                    nc.sync.dma_start(yt[n * P : (n + 1) * P, cs], row[:, cs])

            # Warmup: n=0 and n=1 interleaved t-outer, so the PE has ~3us
            # of work per arriving X t-chunk and never outruns the load.
            warm_rows = {
                n: row_pool.tile([P, T], f32, tag="row", name="row")
                for n in (0, 1)
            }
            for t in range(TT):
                for n in (0, 1):
                    ps = psum_pool.tile([P, 512], f32, tag="ps", name="ps")
                    mm_group(n, t, ps)
                    drain(n, t, ps, warm_rows[n])
            for n in (0, 1):
                store_row(n, warm_rows[n], nchunks=2)

            # Steady state: weight-stationary, each W slice feeds 8 moving
            # t-chunks ([k, t] order inside mm_group column: emit per k).
            for n in range(2, NN - 1):
                row = row_pool.tile([P, T], f32, tag="row", name="row")
                pss = [
                    psum_pool.tile([P, 512], f32, tag="ps", name="ps")
                    for _ in range(TT)
                ]
                for k in range(KB):
                    for t in range(TT):
                        nc.tensor.matmul(
                            pss[t],
                            lhsT=wb_sb[:, n, k, :],
                            rhs=xb_sb[:, k, t * 512 : (t + 1) * 512],
                            start=(k == 0),
                            stop=False,
                        )
                for t in range(TT):
                    nc.tensor.matmul(
                        pss[t],
                        lhsT=w8_sb[:, n, :, :],
                        rhs=x8_sb[:, :, t * 512 : (t + 1) * 512],
                        start=False,
                        stop=True,
                        perf_mode=DR,
                    )
                for t in range(TT):
                    drain(n, t, pss[t], row)
                store_row(n, row, nchunks=2)

            # Last n-slice runs t-outer with inline drain + per-chunk store,
            # so only one chunk's drain+store remains after the last matmul.
            n = NN - 1
            row = row_pool.tile([P, T], f32, tag="row", name="row")
            for t in range(TT - 1):
                ps = psum_pool.tile([P, 512], f32, tag="ps", name="ps")
                mm_group(n, t, ps)
                drain(n, t, ps, row)
                ts = slice(t * 512, (t + 1) * 512)
                nc.sync.dma_start(yt[n * P : (n + 1) * P, ts], row[:, ts])
            # Final chunk as 384+128 sub-region groups in one bank: the big
            # piece's drain+store overlap the micro piece's matmuls, so only
            # a [128,128] drain and a 64KB store trail the last matmul.
            base = (TT - 1) * 512
            ps = psum_pool.tile([P, 512], f32, tag="ps", name="ps")
            tsA = slice(base, base + 384)
            tsB = slice(base + 384, base + 512)
            mm_piece(n, tsA, ps[:, 0:384])
            nc.vector.tensor_scalar(
                row[:, tsA], ps[:, 0:384], cvec_sb[:, 0:1],
                biasv_sb[:, n : n + 1], mult, add
            )
            nc.sync.dma_start(yt[n * P : (n + 1) * P, tsA], row[:, tsA])
            mm_piece(n, tsB, ps[:, 384:512])
            nc.scalar.activation(
                row[:, tsB], ps[:, 384:512], Identity,
                bias=biasv_sb[:, n : n + 1], scale=cvec_sb[:, 0:1],
            )
            nc.sync.dma_start(yt[n * P : (n + 1) * P, tsB], row[:, tsB])
    nc.compile()
    return nc


def _get_nc():
    if "nc" not in _NC_CACHE:
        _NC_CACHE["nc"] = _build()
    return _NC_CACHE["nc"]


def _pack_x(a, ks):
    """[ks*P, T] -> [P, ks, T] contiguous (partition-major k-slices)."""
    return np.ascontiguousarray(a.reshape(ks, P, T).transpose(1, 0, 2))


def _pack_w(a, ks):
    """[ks*P, N3] -> [NN, P, ks, P] contiguous (n-block-major)."""
    return np.ascontiguousarray(
        a.reshape(ks, P, NN, P).transpose(2, 1, 0, 3)
    )


def _prep_in_maps(inputs):
    x = np.asarray(inputs["x"], dtype=np.float32)
    weight = np.asarray(inputs["weight"], dtype=np.float32)
    bias = np.asarray(inputs["bias"], dtype=np.float32)
    aq = np.asarray(inputs["A_q_pool"], dtype=np.float32)
    bq = np.asarray(inputs["B_q_pool"], dtype=np.float32)
    av = np.asarray(inputs["A_v_pool"], dtype=np.float32)
    bv = np.asarray(inputs["B_v_pool"], dtype=np.float32)
    idx = np.asarray(inputs["idx"]).reshape(B, -1)[:, 0].astype(np.int64)

    wt64 = weight.T.astype(np.float64)  # [D, N3]
    biasv = np.ascontiguousarray(bias.reshape(NN, P).T)  # [P, NN]
    xts = x.transpose(0, 2, 1)  # [B, D, T] strided view
    split = KB * P

    in_maps = []
    for b in range(B):
        i = int(idx[b])
        weff = wt64.copy()
        weff[:, :D] += SCALE * (aq[i].astype(np.float64) @ bq[i].astype(np.float64))
        weff[:, N3 - D:] += SCALE * (av[i].astype(np.float64) @ bv[i].astype(np.float64))
        weff = weff.astype(np.float32)

        xt = np.ascontiguousarray(xts[b])  # [D, T] f32
        sx8 = 2.0 ** math.floor(math.log2(FP8_TARGET / float(np.abs(xt).max())))
        sw8 = 2.0 ** math.floor(math.log2(FP8_TARGET / float(np.abs(weff).max())))
        s8 = sx8 * sw8

        in_maps.append({
            "xb": _pack_x(xt[:split].astype(BF16), KB),
            "x8": _pack_x((xt[split:] * sx8).astype(E4), 2),
            "wb": _pack_w((weff[:split] * s8).astype(BF16), KB),
            "w8": _pack_w((weff[split:] * sw8).astype(E4), 2),
            "biasv": biasv,
            "cvec": np.full((P, 1), 1.0 / s8, dtype=np.float32),
        })
    return in_maps


def _postprocess(res):
    return np.stack([r["yt"].T for r in res.results], axis=0)


def _run(in_maps, trace=False, **kwargs):
    from concourse.bass_utils import run_bass_kernel_spmd

    nc = _get_nc()
    return run_bass_kernel_spmd(
        nc, in_maps, core_ids=list(range(B)), trace=trace, **kwargs
    )


def kernel(**inputs):
    res = _run(_prep_in_maps(inputs), trace=False)
    return _postprocess(res)
